# revision 10
# baseline (speedup 1.0000x reference)
"""MultiHeadAttention Trainium2 kernel.

Sharding: 8 cores = 4 batches x 2 head-groups (8 heads each).
Each core computes, for its (batch b, head group gi):
  Q = q[b] @ Wq[:, gi*512:+512] + bq_g        (and same fc applied to k, v)
  per head: softmax(QK^T/8 with mask) @ V
  partial_out = attn @ Wo[gi*512:+512, :]
Host sums the two partial outputs per batch and adds b_o.

Device layout notes (per core):
  - inputs arrive TRANSPOSED: xT [1024, seq] so projections need no transposes
  - Q^T, K^T stored [128, 4, seq] bf16 (partition = d within d-tile; head g
    occupies partitions 64*(g%2).. of d-tile g//2 -> natural head pairing)
  - scores computed transposed S^T[sk, sq] with two heads packed in the PE
    array via tile_position row tiling (K=64 each)
  - softmax without max-subtraction (scores bounded ~|6| after 1/8 scaling)
  - mask applied multiplicatively AFTER exp (notmask in {0,1} bf16)
  - attn@V uses lhsT=[V_head | ones] (M=65): row 64 accumulates the softmax
    denominator for free
  - normalize via reciprocal_approx_fast + DMA partition-broadcast
"""

import sys

import numpy as np
import ml_dtypes

try:
    import concourse.bass as bass  # noqa: F401
except ImportError:  # pragma: no cover
    for _p in ("/opt/trn_rl_repo", "/root/.axon_site/_ro/trn_rl_repo"):
        if _p not in sys.path:
            sys.path.insert(0, _p)
    import concourse.bass as bass  # noqa: F401

import concourse.tile as tile
from concourse import bacc, mybir
from concourse.bass_utils import run_bass_kernel_spmd

BF16 = ml_dtypes.bfloat16

D_MODEL = 1024
N_HEADS = 16
BATCH = 4
SEQ = 2048
DH = 64           # head dim
HG = 8            # heads per core
DG = HG * DH      # 512, projected dim per core

F32 = mybir.dt.float32
F32R = mybir.dt.float32r
BF16D = mybir.dt.bfloat16


def build_nc(seq=SEQ):
    """Build the per-core SPMD Bass program."""
    assert seq % 512 == 0
    NT = seq // 128       # sk tiles
    NC_ = seq // 512      # sq chunks
    NST = seq // 128      # s tiles for V / out rows

    nc = bacc.Bacc(None, target_bir_lowering=False)

    xqT = nc.dram_tensor("xqT", [D_MODEL, seq], F32R, kind="ExternalInput")
    xkT = nc.dram_tensor("xkT", [D_MODEL, seq], F32R, kind="ExternalInput")
    xvT = nc.dram_tensor("xvT", [D_MODEL, seq], F32R, kind="ExternalInput")
    wq = nc.dram_tensor("wq", [D_MODEL, DG], F32R, kind="ExternalInput")
    bq = nc.dram_tensor("bq", [DG], F32, kind="ExternalInput")
    wo = nc.dram_tensor("wo", [DG, D_MODEL], BF16D, kind="ExternalInput")
    # notmask, transposed + tiled: [pair, c, t, p(sk in tile), h(in pair), j(sq in chunk)]
    nm = nc.dram_tensor("nm", [4, NC_, NT, 128, 2, 512], BF16D, kind="ExternalInput")
    out = nc.dram_tensor("out", [seq, D_MODEL], F32, kind="ExternalOutput")

    EXP = mybir.ActivationFunctionType.Exp
    LN = mybir.ActivationFunctionType.Ln
    IDENT = mybir.ActivationFunctionType.Identity

    with tile.TileContext(nc) as tc:
        with tc.tile_pool(name="persist", bufs=1) as persist:
            qt_sb = persist.tile([128, 4, seq], BF16D, name="qt_sb")
            kt_sb = persist.tile([128, 4, seq], BF16D, name="kt_sb")
            v_sb = persist.tile([128, NST, HG, DH + 1], BF16D, name="v_sb")
            wo_sb = persist.tile([128, 4, D_MODEL], BF16D, name="wo_sb")
            attnT = persist.tile([128, 4, seq], BF16D, name="attnT")
            bq_sb = persist.tile([128, 4], F32, name="bq_sb")
            bqrep = persist.tile([128, HG, DH], F32, name="bqrep")

            # ---------------- Phase A: projections ----------------
            with tc.tile_pool(name="xpool", bufs=12) as xpool, \
                 tc.tile_pool(name="wqpool", bufs=1) as wqp, \
                 tc.tile_pool(name="projps", bufs=2, space="PSUM") as projps:

                wq_sb = wqp.tile([128, 8, DG], F32R, name="wq_sb")
                nc.sync.dma_start(out=wq_sb, in_=wq.rearrange("(n p) m -> p n m", p=128))
                nc.sync.dma_start(out=bq_sb, in_=bq.rearrange("(t p) -> p t", p=128))
                _bqap = bq[:].rearrange("(g e) -> g e", g=HG)
                nc.gpsimd.dma_start(out=bqrep, in_=bass.AP(
                    tensor=_bqap.tensor, offset=_bqap.offset,
                    ap=[[0, 128]] + [list(d) for d in _bqap.ap]))
                nc.sync.dma_start(out=wo_sb, in_=wo.rearrange("(n p) m -> p n m", p=128))
                nc.vector.memset(v_sb[:, :, :, DH:DH + 1], 1.0)

                def load_x(xh, pref):
                    ts_ = []
                    for db in range(8):
                        xt = xpool.tile([128, seq], F32R, name=f"{pref}{db}", tag="x")
                        nc.sync.dma_start(out=xt, in_=xh[db * 128:(db + 1) * 128, :])
                        ts_.append(xt)
                    return ts_

                # Q^T and K^T: out[d_tile, sq] accumulated over D blocks
                for xh, dst in ((xqT, qt_sb), (xkT, kt_sb)):
                    xts = load_x(xh, "xq" if dst is qt_sb else "xk")
                    for dt in range(4):
                        for h0 in range(0, seq, 1024):
                            cw = min(1024, seq - h0)
                            ps = projps.tile([128, cw], F32, name="pps", tag="pps")
                            for db in range(8):
                                for j0 in range(0, cw, 512):
                                    nc.tensor.matmul(
                                        ps[:, j0:j0 + 512],
                                        wq_sb[:, db, dt * 128:(dt + 1) * 128],
                                        xts[db][:, h0 + j0: h0 + j0 + 512],
                                        start=(db == 0), stop=(db == 7),
                                    )
                            nc.scalar.activation(
                                dst[:, dt, h0:h0 + cw], ps,
                                IDENT, bias=bq_sb[:, dt:dt + 1], scale=1.0,
                            )
                # V: out[s_tile, d] accumulated over D blocks
                xts = load_x(xvT, "xv")
                for st in range(NST):
                    ps = projps.tile([128, DG], F32, name="pps", tag="pps")
                    for db in range(8):
                        nc.tensor.matmul(
                            ps, xts[db][:, st * 128:(st + 1) * 128], wq_sb[:, db, :],
                            start=(db == 0), stop=(db == 7),
                        )
                    nc.vector.tensor_add(
                        v_sb[:, st, :, 0:DH],
                        ps[:, :].rearrange("p (g e) -> p g e", g=HG),
                        bqrep,
                    )

            # ---------------- Phase B: attention ----------------
            with tc.tile_pool(name="nmp", bufs=10) as nmp, \
                 tc.tile_pool(name="probsp", bufs=4) as probsp, \
                 tc.tile_pool(name="rsbp", bufs=2) as rsbp, \
                 tc.tile_pool(name="rrepp", bufs=2) as rrepp, \
                 tc.tile_pool(name="tmpp", bufs=2) as tmpp, \
                 tc.tile_pool(name="osbp", bufs=3) as osbp, \
                 tc.tile_pool(name="spairp", bufs=2, space="PSUM") as spairp, \
                 tc.tile_pool(name="accp", bufs=1, space="PSUM") as accp, \
                 tc.tile_pool(name="outpsp", bufs=2, space="PSUM") as outpsp:

                for c in range(NC_):
                    cs = slice(c * 512, (c + 1) * 512)
                    for pr in range(4):
                        acc = accp.tile([DH + 1, 2, 512], F32, name="acc", tag="acc")
                        for t in range(NT):
                            nmt = nmp.tile([128, 2, 512], BF16D, name="nmt", tag="nmt")
                            nc.sync.dma_start(out=nmt, in_=nm[pr, c, t])
                            spair = spairp.tile([128, 2, 512], F32, name="spair", tag="spair")
                            tc_cols = slice(t * 128, (t + 1) * 128)
                            nc.tensor.matmul(
                                spair[:, 0, :], kt_sb[0:64, pr, tc_cols],
                                qt_sb[0:64, pr, cs], start=True, stop=True,
                                tile_position=(0, 0),
                            )
                            nc.tensor.matmul(
                                spair[:, 1, :], kt_sb[64:128, pr, tc_cols],
                                qt_sb[64:128, pr, cs], start=True, stop=True,
                                tile_position=(64, 0),
                            )
                            probs = probsp.tile([128, 2, 512], BF16D, name="probs", tag="probs")
                            nc.scalar.activation(probs, spair, EXP, scale=0.125)
                            nc.vector.tensor_mul(probs, probs, nmt)
                            for h in range(2):
                                nc.tensor.matmul(
                                    acc[:, h, :], v_sb[:, t, 2 * pr + h, :],
                                    probs[:, h, :], start=(t == 0), stop=(t == NT - 1),
                                )
                        # normalize this pair's chunk
                        lnt = rsbp.tile([1, 2, 512], F32, name="lnt", tag="lnt")
                        nc.scalar.activation(lnt, acc[DH:DH + 1, :, :], LN, scale=1.0)
                        rsb = rsbp.tile([1, 2, 512], F32, name="rsb", tag="rsb")
                        nc.scalar.activation(rsb, lnt, EXP, scale=-1.0)
                        rrep = rrepp.tile([64, 2, 512], F32, name="rrep", tag="rrep")
                        nc.gpsimd.partition_broadcast(rrep, rsb)
                        tmpt = tmpp.tile([64, 2, 512], BF16D, name="tmpt", tag="tmpt")
                        nc.vector.tensor_mul(tmpt, acc[0:DH, :, :], rrep)
                        nc.sync.dma_start(out=attnT[0:64, pr, cs], in_=tmpt[:, 0, :])
                        nc.sync.dma_start(out=attnT[64:128, pr, cs], in_=tmpt[:, 1, :])
                    # output projection for this sq chunk
                    for k in range(4):
                        row = c * 512 + k * 128
                        for dch in range(2):
                            ops = outpsp.tile([128, 512], F32, name="ops", tag="ops")
                            for pr in range(4):
                                nc.tensor.matmul(
                                    ops, attnT[:, pr, row:row + 128],
                                    wo_sb[:, pr, dch * 512:(dch + 1) * 512],
                                    start=(pr == 0), stop=(pr == 3),
                                )
                            osb = osbp.tile([128, 512], F32, name="osb", tag="osb")
                            nc.vector.tensor_copy(osb, ops)
                            nc.sync.dma_start(
                                out=out[row:row + 128, dch * 512:(dch + 1) * 512], in_=osb
                            )

    nc.compile()
    return nc


_NC_CACHE = {}


def _get_nc(seq=SEQ):
    if seq not in _NC_CACHE:
        _NC_CACHE[seq] = build_nc(seq)
    return _NC_CACHE[seq]


def make_core_inputs(q, k, v, mask, W_q, b_q, W_o, seq=SEQ):
    """Build the 8 per-core input maps (host-side shard + layout)."""
    NT = seq // 128
    NC_ = seq // 512
    in_maps = []
    notm_all = (~np.asarray(mask)).astype(BF16)  # [B, 16, sq, sk]
    for core in range(8):
        b, gi = divmod(core, 2)
        cols = slice(gi * DG, (gi + 1) * DG)
        xqT = np.ascontiguousarray(np.asarray(q[b], np.float32).T)
        xkT = np.ascontiguousarray(np.asarray(k[b], np.float32).T)
        xvT = np.ascontiguousarray(np.asarray(v[b], np.float32).T)
        wqc = np.ascontiguousarray(np.asarray(W_q, np.float32)[:, cols])
        bqc = np.ascontiguousarray(np.asarray(b_q, np.float32)[cols])
        woc = np.ascontiguousarray(np.asarray(W_o, np.float32)[cols, :]).astype(BF16)
        nmc = notm_all[b, gi * HG:(gi + 1) * HG]  # [8, sq, sk] bf16
        # -> [pair, c, t, p, h, j]
        nmc = np.ascontiguousarray(
            nmc.reshape(4, 2, NC_, 512, NT, 128).transpose(0, 2, 4, 5, 1, 3)
        )
        in_maps.append({
            "xqT": xqT, "xkT": xkT, "xvT": xvT,
            "wq": wqc, "bq": bqc, "wo": woc, "nm": nmc,
        })
    return in_maps


def kernel(q, k, v, mask, W_q, b_q, W_o, b_o):
    nc = _get_nc(SEQ)
    in_maps = make_core_inputs(q, k, v, mask, W_q, b_q, W_o, SEQ)
    res = run_bass_kernel_spmd(nc, in_maps, core_ids=list(range(8)))
    out = np.empty((BATCH, SEQ, D_MODEL), np.float32)
    bo = np.asarray(b_o, np.float32)
    for b in range(BATCH):
        out[b] = res.results[2 * b]["out"] + res.results[2 * b + 1]["out"] + bo
    return out
